# revision 16
# baseline (speedup 1.0000x reference)
"""BoxFilter (9x9 box sum with edge clamping) on 8 Trainium2 NeuronCores.

Reference semantics (B, C, H, W fp32, r=4):
    out = diff_y(cumsum_W(diff_x(cumsum_H(x))))
i.e. a separable 9-wide box *sum* along H then W, with windows truncated at
the image borders.

Strategy:
  - Shard data-parallel over batch: B=8 -> one (3, 1080, 1920) image per core.
  - Per core, 27 tiles (3 channels x 9 blocks of 120 output rows). Each tile
    loads 128 input rows (+-4 halo) x full W.
  - W-direction box: one DVE tensor_tensor_scan implementing
        S[w] = S[w-1] + x[w+4] - x[w-5]
    over a zero-padded row. The row is padded with 2r+1 zeros on the left and
    r on the right, and the scan starts r steps early from initial=0 so the
    window warm-up happens inside the scan (no separate init reduce).
  - H-direction box: TensorE matmul with a constant 0/1 banded matrix
    [K=128, M=120] (one variant each for top / interior / bottom blocks).
  - PSUM -> SBUF via ScalarE copy; loads on the SP HWDGE ring, stores on the
    ACT HWDGE ring.
"""

import sys

if "/opt/trn_rl_repo" not in sys.path:
    sys.path.insert(0, "/opt/trn_rl_repo")

import numpy as np

B, C, H, W = 8, 3, 1080, 1920
R = 4
BLK = 120          # output rows per tile
NBLK = H // BLK    # 9
LPAD = 2 * R + 1   # 9 left zeros
XW = LPAD + W + R  # padded row width (1933)
SCN = W + R        # scan length (1924); outputs [R:] are S[0..W-1]
N_CHUNKS = (W + 511) // 512  # matmul N<=512 fp32 (PSUM bank)


def _band_matrices() -> np.ndarray:
    """[128, 3*BLK] fp32: the three 0/1 banded H-box matrices, side by side.

    out[m, n] = sum_k band[k, m] * in[k, n]; column m holds the taps for
    output row m of the block.
    """
    b0 = np.zeros((128, BLK), np.float32)   # first block: rows 0..127 loaded
    b1 = np.zeros((128, BLK), np.float32)   # interior: rows h0-4..h0+123
    b2 = np.zeros((128, BLK), np.float32)   # last block: rows H-128..H-1
    for m in range(BLK):
        b0[max(0, m - R): m + R + 1, m] = 1.0
        b1[m: m + 2 * R + 1, m] = 1.0
        b2[m + R: min(m + 3 * R, 127) + 1, m] = 1.0
    return np.concatenate([b0, b1, b2], axis=1)


def _build_nc():
    import concourse.tile as tile
    from concourse import bacc, mybir

    f32 = mybir.dt.float32
    nc = bacc.Bacc("TRN2", target_bir_lowering=False, debug=False)
    x_d = nc.dram_tensor("x", [C, H, W], f32, kind="ExternalInput").ap()
    out_d = nc.dram_tensor("out", [C, H, W], f32, kind="ExternalOutput").ap()
    bands_d = nc.inline_tensor(_band_matrices(), name="bands").ap()

    with tile.TileContext(nc) as tc:
        _tile_body(tc, out_d, x_d, bands_d, f32, mybir)
    nc.compile()
    return nc


USE_F32R = True  # float32r matmuls: 1 HW pass instead of fp32's LOW_HIGH 2


def _tile_body(tc, out_d, x_d, bands_d, f32, mybir):
    nc = tc.nc
    add = mybir.AluOpType.add
    sub = mybir.AluOpType.subtract
    f32r = mybir.dt.float32r
    mm_dt = f32r if USE_F32R else f32

    with (
        tc.tile_pool(name="bands", bufs=1) as bands_pool,
        tc.tile_pool(name="xp", bufs=6) as xpool,
        tc.tile_pool(name="wb", bufs=5) as wpool,
        tc.tile_pool(name="ot", bufs=5) as opool,
        tc.tile_pool(name="ps", bufs=2, space="PSUM") as pspool,
    ):
        bands = bands_pool.tile([128, 3 * BLK], mm_dt)
        first = True

        for c in range(C):
            for t in range(NBLK):
                h0 = t * BLK
                if t == 0:
                    r0, bi = 0, 0
                elif t == NBLK - 1:
                    r0, bi = H - 128, 2
                else:
                    r0, bi = h0 - R, 1

                xp = xpool.tile([128, XW], f32)
                nc.gpsimd.memset(xp[:, 0:LPAD], 0.0)
                nc.gpsimd.memset(xp[:, LPAD + W: XW], 0.0)
                nc.sync.dma_start(
                    out=xp[:, LPAD: LPAD + W], in_=x_d[c, r0: r0 + 128, :]
                )
                if first:
                    # bands aren't needed until the first matmul; don't let
                    # their DMA delay the first tile load. (0/1 values are
                    # exact in f32r, so the bitcast is value-preserving.)
                    nc.sync.dma_start(
                        out=bands[:, :], in_=bands_d[:, :].bitcast(bands.dtype)
                    )
                    first = False

                # scan t=0..SCN-1: state = (xp[t+LPAD-R] ... ) computing
                # S[w] = S[w-1] + x[w+4] - x[w-5] from w=-R with state 0;
                # wb[:, R:] holds S[0..W-1]
                wb = wpool.tile([128, SCN], mm_dt)
                nc.vector.tensor_tensor_scan(
                    out=wb[:, :],
                    data0=xp[:, LPAD: LPAD + SCN],
                    data1=xp[:, 0:SCN],
                    initial=0.0,
                    op0=add,
                    op1=sub,
                )

                band = bands[:, bi * BLK: (bi + 1) * BLK]
                ps = pspool.tile([BLK, N_CHUNKS * 512], f32)
                for j in range(N_CHUNKS):
                    n0 = j * 512
                    nw = min(512, W - n0)
                    nc.tensor.matmul(
                        out=ps[:, n0: n0 + nw],
                        lhsT=band,
                        rhs=wb[:, R + n0: R + n0 + nw],
                        start=True,
                        stop=True,
                    )

                ot = opool.tile([BLK, W], f32)
                nc.scalar.copy(out=ot[:, :], in_=ps[:, 0:W])
                # stores on the ACT HWDGE ring so they don't head-of-line
                # block loads on the SP ring
                nc.scalar.dma_start(out=out_d[c, h0: h0 + BLK, :], in_=ot[:, :])


_NC = None


def _get_nc():
    global _NC
    if _NC is None:
        _NC = _build_nc()
    return _NC


def run(x: np.ndarray, trace: bool = False, trace_cores=None):
    """Run the kernel on all 8 cores. Returns (out, BassKernelResults)."""
    from concourse.bass_utils import run_bass_kernel_spmd

    nc = _get_nc()
    x = np.ascontiguousarray(np.asarray(x, dtype=np.float32))
    assert x.shape == (B, C, H, W), x.shape
    in_maps = [{"x": x[b]} for b in range(B)]
    if trace and trace_cores is None:
        trace_cores = [0, 7]
    res = run_bass_kernel_spmd(
        nc, in_maps, core_ids=list(range(B)), trace=trace, trace_cores=trace_cores
    )
    out = np.stack([res.results[b]["out"] for b in range(B)], axis=0)
    return out, res


def kernel(x: np.ndarray, r) -> np.ndarray:
    assert int(np.asarray(r)) == R, f"kernel hardcodes r={R}, got {r}"
    out, _ = run(x, trace=False)
    return out


# revision 17
# speedup vs baseline: 1.1327x; 1.1327x over previous
"""BoxFilter (9x9 box sum with edge clamping) on 8 Trainium2 NeuronCores.

Reference semantics (B, C, H, W fp32, r=4):
    out = diff_y(cumsum_W(diff_x(cumsum_H(x))))
i.e. a separable 9-wide box *sum* along H then W, with windows truncated at
the image borders.

Strategy:
  - Shard data-parallel over batch: B=8 -> one (3, 1080, 1920) image per core.
  - Per core, 27 tiles (3 channels x 9 blocks of 120 output rows). Each tile
    loads 128 input rows (+-4 halo) x full W.
  - W-direction box: one DVE tensor_tensor_scan implementing
        S[w] = S[w-1] + x[w+4] - x[w-5]
    over a zero-padded row. The row is padded with 2r+1 zeros on the left and
    r on the right, and the scan starts r steps early from initial=0 so the
    window warm-up happens inside the scan (no separate init reduce).
  - H-direction box: TensorE matmul with a constant 0/1 banded matrix
    [K=128, M=120] (one variant each for top / interior / bottom blocks).
  - PSUM -> SBUF via ScalarE copy; loads on the SP HWDGE ring, stores on the
    ACT HWDGE ring.
"""

import sys

if "/opt/trn_rl_repo" not in sys.path:
    sys.path.insert(0, "/opt/trn_rl_repo")

import numpy as np

B, C, H, W = 8, 3, 1080, 1920
R = 4
BLK = 120          # output rows per tile
NBLK = H // BLK    # 9
LPAD = 2 * R + 1   # 9 left zeros
XW = LPAD + W + R  # padded row width (1933)
SCN = W + R        # scan length (1924); outputs [R:] are S[0..W-1]
N_CHUNKS = (W + 511) // 512  # matmul N<=512 fp32 (PSUM bank)


def _band_matrices() -> np.ndarray:
    """[128, 3*BLK] fp32: the three 0/1 banded H-box matrices, side by side.

    out[m, n] = sum_k band[k, m] * in[k, n]; column m holds the taps for
    output row m of the block.
    """
    b0 = np.zeros((128, BLK), np.float32)   # first block: rows 0..127 loaded
    b1 = np.zeros((128, BLK), np.float32)   # interior: rows h0-4..h0+123
    b2 = np.zeros((128, BLK), np.float32)   # last block: rows H-128..H-1
    for m in range(BLK):
        b0[max(0, m - R): m + R + 1, m] = 1.0
        b1[m: m + 2 * R + 1, m] = 1.0
        b2[m + R: min(m + 3 * R, 127) + 1, m] = 1.0
    return np.concatenate([b0, b1, b2], axis=1)


def _build_nc():
    import concourse.tile as tile
    from concourse import bacc, mybir

    f32 = mybir.dt.float32
    nc = bacc.Bacc("TRN2", target_bir_lowering=False, debug=False)
    x_d = nc.dram_tensor("x", [C, H, W], f32, kind="ExternalInput").ap()
    out_d = nc.dram_tensor("out", [C, H, W], f32, kind="ExternalOutput").ap()
    bands_d = nc.inline_tensor(_band_matrices(), name="bands").ap()

    with tile.TileContext(nc) as tc:
        _tile_body(tc, out_d, x_d, bands_d, f32, mybir)
    nc.compile()
    return nc


# float32r matmuls would run 1 HW pass instead of fp32's LOW_HIGH 2 and save
# ~4us, but round the scan output to ~12 mantissa bits (absmax err 6.6e-3,
# ~50x the reference's own fp32 envelope of 1.3e-4). Not worth the risk
# against an envelope-based correctness gate.
USE_F32R = False


def _tile_body(tc, out_d, x_d, bands_d, f32, mybir):
    nc = tc.nc
    add = mybir.AluOpType.add
    sub = mybir.AluOpType.subtract
    f32r = mybir.dt.float32r
    mm_dt = f32r if USE_F32R else f32

    with (
        tc.tile_pool(name="bands", bufs=1) as bands_pool,
        tc.tile_pool(name="xp", bufs=6) as xpool,
        tc.tile_pool(name="wb", bufs=5) as wpool,
        tc.tile_pool(name="ot", bufs=5) as opool,
        tc.tile_pool(name="ps", bufs=2, space="PSUM") as pspool,
    ):
        bands = bands_pool.tile([128, 3 * BLK], mm_dt)
        first = True

        for c in range(C):
            for t in range(NBLK):
                h0 = t * BLK
                if t == 0:
                    r0, bi = 0, 0
                elif t == NBLK - 1:
                    r0, bi = H - 128, 2
                else:
                    r0, bi = h0 - R, 1

                xp = xpool.tile([128, XW], f32)
                nc.gpsimd.memset(xp[:, 0:LPAD], 0.0)
                nc.gpsimd.memset(xp[:, LPAD + W: XW], 0.0)
                nc.sync.dma_start(
                    out=xp[:, LPAD: LPAD + W], in_=x_d[c, r0: r0 + 128, :]
                )
                if first:
                    # bands aren't needed until the first matmul; don't let
                    # their DMA delay the first tile load. (0/1 values are
                    # exact in f32r, so the bitcast is value-preserving.)
                    nc.sync.dma_start(
                        out=bands[:, :], in_=bands_d[:, :].bitcast(bands.dtype)
                    )
                    first = False

                # scan t=0..SCN-1: state = (xp[t+LPAD-R] ... ) computing
                # S[w] = S[w-1] + x[w+4] - x[w-5] from w=-R with state 0;
                # wb[:, R:] holds S[0..W-1]
                wb = wpool.tile([128, SCN], mm_dt)
                nc.vector.tensor_tensor_scan(
                    out=wb[:, :],
                    data0=xp[:, LPAD: LPAD + SCN],
                    data1=xp[:, 0:SCN],
                    initial=0.0,
                    op0=add,
                    op1=sub,
                )

                band = bands[:, bi * BLK: (bi + 1) * BLK]
                ps = pspool.tile([BLK, N_CHUNKS * 512], f32)
                for j in range(N_CHUNKS):
                    n0 = j * 512
                    nw = min(512, W - n0)
                    nc.tensor.matmul(
                        out=ps[:, n0: n0 + nw],
                        lhsT=band,
                        rhs=wb[:, R + n0: R + n0 + nw],
                        start=True,
                        stop=True,
                    )

                ot = opool.tile([BLK, W], f32)
                nc.scalar.copy(out=ot[:, :], in_=ps[:, 0:W])
                # stores on the ACT HWDGE ring so they don't head-of-line
                # block loads on the SP ring
                nc.scalar.dma_start(out=out_d[c, h0: h0 + BLK, :], in_=ot[:, :])


_NC = None


def _get_nc():
    global _NC
    if _NC is None:
        _NC = _build_nc()
    return _NC


def run(x: np.ndarray, trace: bool = False, trace_cores=None):
    """Run the kernel on all 8 cores. Returns (out, BassKernelResults)."""
    from concourse.bass_utils import run_bass_kernel_spmd

    nc = _get_nc()
    x = np.ascontiguousarray(np.asarray(x, dtype=np.float32))
    assert x.shape == (B, C, H, W), x.shape
    in_maps = [{"x": x[b]} for b in range(B)]
    if trace and trace_cores is None:
        trace_cores = [0, 7]
    res = run_bass_kernel_spmd(
        nc, in_maps, core_ids=list(range(B)), trace=trace, trace_cores=trace_cores
    )
    out = np.stack([res.results[b]["out"] for b in range(B)], axis=0)
    return out, res


def kernel(x: np.ndarray, r) -> np.ndarray:
    assert int(np.asarray(r)) == R, f"kernel hardcodes r={R}, got {r}"
    out, _ = run(x, trace=False)
    return out


# revision 22
# speedup vs baseline: 1.1533x; 1.0181x over previous
"""BoxFilter (9x9 box sum with edge clamping) on 8 Trainium2 NeuronCores.

Reference semantics (B, C, H, W fp32, r=4):
    out = diff_y(cumsum_W(diff_x(cumsum_H(x))))
i.e. a separable 9-wide box *sum* along H then W, with windows truncated at
the image borders.

Strategy:
  - Shard data-parallel over batch: B=8 -> one (3, 1080, 1920) image per core.
  - Per core, 27 tiles (3 channels x 9 blocks of 120 output rows). Each tile
    loads 128 input rows (+-4 halo) x full W.
  - W-direction box: one DVE tensor_tensor_scan implementing
        S[w] = S[w-1] + x[w+4] - x[w-5]
    over a zero-padded row. The row is padded with 2r+1 zeros on the left and
    r on the right, and the scan starts r steps early from initial=0 so the
    window warm-up happens inside the scan (no separate init reduce).
  - H-direction box: TensorE matmul with a constant 0/1 banded matrix
    [K=128, M=120] (one variant each for top / interior / bottom blocks).
  - PSUM -> SBUF via ScalarE copy; loads on the SP HWDGE ring, stores on the
    ACT HWDGE ring.
"""

import sys

if "/opt/trn_rl_repo" not in sys.path:
    sys.path.insert(0, "/opt/trn_rl_repo")

import numpy as np

B, C, H, W = 8, 3, 1080, 1920
R = 4
BLK = 120          # output rows per tile
NBLK = H // BLK    # 9
LPAD = 2 * R + 1   # 9 left zeros
XW = LPAD + W + R  # padded row width (1933)
SCN = W + R        # scan length (1924); outputs [R:] are S[0..W-1]
N_CHUNKS = (W + 511) // 512  # matmul N<=512 fp32 (PSUM bank)


def _band_matrices() -> np.ndarray:
    """[128, 3*BLK] fp32: the three 0/1 banded H-box matrices, side by side.

    out[m, n] = sum_k band[k, m] * in[k, n]; column m holds the taps for
    output row m of the block.
    """
    b0 = np.zeros((128, BLK), np.float32)   # first block: rows 0..127 loaded
    b1 = np.zeros((128, BLK), np.float32)   # interior: rows h0-4..h0+123
    b2 = np.zeros((128, BLK), np.float32)   # last block: rows H-128..H-1
    for m in range(BLK):
        b0[max(0, m - R): m + R + 1, m] = 1.0
        b1[m: m + 2 * R + 1, m] = 1.0
        b2[m + R: min(m + 3 * R, 127) + 1, m] = 1.0
    return np.concatenate([b0, b1, b2], axis=1)


def _build_nc():
    import concourse.tile as tile
    from concourse import bacc, mybir

    f32 = mybir.dt.float32
    nc = bacc.Bacc("TRN2", target_bir_lowering=False, debug=False)
    x_d = nc.dram_tensor("x", [C, H, W], f32, kind="ExternalInput").ap()
    out_d = nc.dram_tensor("out", [C, H, W], f32, kind="ExternalOutput").ap()
    bands_d = nc.inline_tensor(_band_matrices(), name="bands").ap()

    with tile.TileContext(nc) as tc:
        _tile_body(tc, out_d, x_d, bands_d, f32, mybir)
    nc.compile()
    return nc


# float32r matmuls would run 1 HW pass instead of fp32's LOW_HIGH 2 and save
# ~4us, but round the scan output to ~12 mantissa bits (absmax err 6.6e-3,
# ~50x the reference's own fp32 envelope of 1.3e-4). Not worth the risk
# against an envelope-based correctness gate.
USE_F32R = False


def _tile_body(tc, out_d, x_d, bands_d, f32, mybir):
    nc = tc.nc
    add = mybir.AluOpType.add
    sub = mybir.AluOpType.subtract
    f32r = mybir.dt.float32r
    mm_dt = f32r if USE_F32R else f32

    with (
        tc.tile_pool(name="bands", bufs=1) as bands_pool,
        tc.tile_pool(name="xp", bufs=7) as xpool,
        tc.tile_pool(name="wb", bufs=6) as wpool,
        tc.tile_pool(name="ot", bufs=5) as opool,
        tc.tile_pool(name="ps", bufs=8, space="PSUM") as pspool,
    ):
        bands = bands_pool.tile([128, 3 * BLK], mm_dt)
        first = True

        for c in range(C):
            for t in range(NBLK):
                h0 = t * BLK
                if t == 0:
                    r0, bi = 0, 0
                elif t == NBLK - 1:
                    r0, bi = H - 128, 2
                else:
                    r0, bi = h0 - R, 1

                xp = xpool.tile([128, XW], f32)
                nc.gpsimd.memset(xp[:, 0:LPAD], 0.0)
                nc.gpsimd.memset(xp[:, LPAD + W: XW], 0.0)
                nc.sync.dma_start(
                    out=xp[:, LPAD: LPAD + W], in_=x_d[c, r0: r0 + 128, :]
                )
                if first:
                    # bands aren't needed until the first matmul; don't let
                    # their DMA delay the first tile load. (0/1 values are
                    # exact in f32r, so the bitcast is value-preserving.)
                    nc.sync.dma_start(
                        out=bands[:, :], in_=bands_d[:, :].bitcast(bands.dtype)
                    )
                    first = False

                # scan t=0..SCN-1: state = (xp[t+LPAD-R] ... ) computing
                # S[w] = S[w-1] + x[w+4] - x[w-5] from w=-R with state 0;
                # wb[:, R:] holds S[0..W-1]
                wb = wpool.tile([128, SCN], mm_dt)
                nc.vector.tensor_tensor_scan(
                    out=wb[:, :],
                    data0=xp[:, LPAD: LPAD + SCN],
                    data1=xp[:, 0:SCN],
                    initial=0.0,
                    op0=add,
                    op1=sub,
                )

                # one single-bank PSUM tile per 512-col chunk: PE rotates
                # through 8 banks and never waits on a whole-tile evacuation
                band = bands[:, bi * BLK: (bi + 1) * BLK]
                ot = opool.tile([BLK, W], f32)
                for j in range(N_CHUNKS):
                    n0 = j * 512
                    nw = min(512, W - n0)
                    psj = pspool.tile([BLK, 512], f32)
                    nc.tensor.matmul(
                        out=psj[:, 0:nw],
                        lhsT=band,
                        rhs=wb[:, R + n0: R + n0 + nw],
                        start=True,
                        stop=True,
                    )
                    nc.scalar.copy(out=ot[:, n0: n0 + nw], in_=psj[:, 0:nw])
                # stores on the ACT HWDGE ring so they don't head-of-line
                # block loads on the SP ring
                nc.scalar.dma_start(out=out_d[c, h0: h0 + BLK, :], in_=ot[:, :])


_NC = None


def _get_nc():
    global _NC
    if _NC is None:
        _NC = _build_nc()
    return _NC


def run(x: np.ndarray, trace: bool = False, trace_cores=None):
    """Run the kernel on all 8 cores. Returns (out, BassKernelResults)."""
    from concourse.bass_utils import run_bass_kernel_spmd

    nc = _get_nc()
    x = np.ascontiguousarray(np.asarray(x, dtype=np.float32))
    assert x.shape == (B, C, H, W), x.shape
    in_maps = [{"x": x[b]} for b in range(B)]
    if trace and trace_cores is None:
        trace_cores = [0, 7]
    res = run_bass_kernel_spmd(
        nc, in_maps, core_ids=list(range(B)), trace=trace, trace_cores=trace_cores
    )
    out = np.stack([res.results[b]["out"] for b in range(B)], axis=0)
    return out, res


def kernel(x: np.ndarray, r) -> np.ndarray:
    assert int(np.asarray(r)) == R, f"kernel hardcodes r={R}, got {r}"
    out, _ = run(x, trace=False)
    return out
